# revision 51
# baseline (speedup 1.0000x reference)
"""Causal multi-head self-attention on 8 TRN2 NeuronCores.

Problem (hardcoded): x (4, 2048, 1024) f32, W_qkv (3072, 1024), W_o (1024, 1024).
  qkv = x @ W_qkv.T; q,k,v split -> (B,H,T,DK) with H=16, DK=64
  scores = q k^T / 8 + causal mask; attn = softmax; out = (attn v) @ W_o.T
Sharding: core = 2*b + hg (b in 0..3, hg in 0..1 head-groups of 8 heads).
Each core computes a partial out[b] over its 512 attn columns; host sums pairs.

v2 dataflow — one fused instruction stream designed to keep the PE array
HAM-warm (no idle windows) and hide the ACT exp stream:
  - Heads are processed in even/odd pairs. Even heads live on SBUF
    partitions 0-63, odd heads on 64-127, so the two K=64 S^T matmuls of a
    pair auto-row-tile to (0,0)/(64,0) and run concurrently, writing the
    two banks of one [128,1024] PSUM tile; a single wide exp covers both.
  - Causal mask: exp only covers valid widths; the 128-wide diagonal block
    is zeroed post-exp by a bf16 triangular multiply in SBUF (fast DVE mode)
    instead of f32 PSUM adds.
  - QKV projections are computed per 512-wide t-chunk and interleaved into
    the attention stream as PE "filler" (chunks 0-1 in a prologue, later
    chunks funded by earlier attention chunks), so the ACT-bound exp stream
    never leaves the PE idle.
  - AV accumulates v^T x P^T with an extra ones-column per head producing
    softmax denominators on PSUM partitions 64+h; evacuated per pair with a
    lag that crosses pair boundaries (3-deep AV PSUM pool).
  - Normalize: reciprocal_approx_fast on the chunk's [8,512] denominators,
    bf16 K=8 PE broadcast matmul, in-place bf16 DVE multiply. Normalize jobs
    go in the high-priority filler tier; out_proj jobs (attn^T x W_o slices,
    f32 partials to DRAM) in the low tier so they drift into the ACT-bound
    final chunk.
"""

import sys

import numpy as np

sys.path.insert(0, "/opt/trn_rl_repo")

import ml_dtypes  # noqa: E402

from concourse import bacc, bass, mybir, tile  # noqa: E402
from concourse.bass_utils import run_bass_kernel_spmd  # noqa: E402

FP32 = mybir.dt.float32
BF16 = mybir.dt.bfloat16

B, T, D, H, DK = 4, 2048, 1024, 16, 64
NCORES = 8
E = 512          # qkv columns per head-group
NH = 8           # heads per core
P = 128
DCH = D // P     # 8 contraction chunks for the projections
EC = E // P      # 4 e-chunks (head pairs)
NT512 = T // 512
NT128 = T // P
VW = DK + NH     # 72: AV output rows (64 values + 8 one-hot denom rows)
LAG = 3          # AV stream lags the S/exp stream by this many j-steps


def _emit(nc, tc, ctx, xT, wq, wk, wv, wo, tri, sel, out):
    from collections import deque

    consts = ctx.enter_context(tc.tile_pool(name="consts", bufs=1))
    persist = ctx.enter_context(tc.tile_pool(name="persist", bufs=1))

    # ---- persistent SBUF state ----
    xT_sb = persist.tile([P, DCH, T], BF16, name="xT_sb")
    wq_sb = persist.tile([P, DCH, E], BF16, name="wq_sb")
    wk_sb = persist.tile([P, DCH, E], BF16, name="wk_sb")
    wv_sb = persist.tile([P, DCH, E], BF16, name="wv_sb")
    wo_sb = persist.tile([P, EC, D], BF16, name="wo_sb")
    qT = persist.tile([P, EC, T], BF16, name="qT")        # e = ec*128+p
    kT = persist.tile([P, EC, T], BF16, name="kT")
    vt = persist.tile([P, NT128, NH, VW], BF16, name="vt")
    attn = persist.tile([P, EC, T], BF16, name="attn")    # dl = s*128+p
    tri_sb = consts.tile([P, 2, P], BF16, name="tri_sb")
    sel_sb = consts.tile([NH, NH * DK], BF16, name="sel_sb")

    # ---- input DMA: x chunks on sync, weights on gpsimd, late x on scalar,
    # so three DMA streams run in parallel and k-proj deps arrive fastest ----
    for j in range(DCH):
        nc.sync.dma_start(
            xT_sb[:, j, 0:512], xT[j * P : (j + 1) * P, 0:512]
        )
        nc.gpsimd.dma_start(wk_sb[:, j], wk[j * P : (j + 1) * P, :])
    for j in range(DCH):
        nc.gpsimd.dma_start(wq_sb[:, j], wq[j * P : (j + 1) * P, :])
    for j in range(DCH):
        nc.sync.dma_start(
            xT_sb[:, j, 512:1024], xT[j * P : (j + 1) * P, 512:1024]
        )
        nc.gpsimd.dma_start(wv_sb[:, j], wv[j * P : (j + 1) * P, :])
    nc.gpsimd.dma_start(tri_sb[:], tri[:])
    nc.gpsimd.dma_start(sel_sb[:], sel[:])
    for c in (2, 3):
        for j in range(DCH):
            nc.scalar.dma_start(
                xT_sb[:, j, c * 512 : (c + 1) * 512],
                xT[j * P : (j + 1) * P, c * 512 : (c + 1) * 512],
            )
    for s in range(EC):
        nc.gpsimd.dma_start(wo_sb[:, s], wo[s * P : (s + 1) * P, :])

    # head h's ones column sits at DK+h so its denominators land on a
    # distinct psum partition (64+h); other heads' columns there are zero
    nc.vector.memset(vt[:, :, :, DK:], 0.0)
    for hh in range(NH):
        nc.vector.memset(vt[:, :, hh, DK + hh], 1.0)

    # ---- projection generators (each yield ~= one matmul of PE work) ----
    def proj_qk_gen(w_sb, dst, c, pool, ecs=None):
        for ec in (range(EC) if ecs is None else ecs):
            ps = pool.tile([P, 512], FP32, name="ps_g")
            for j in range(DCH):
                nc.tensor.matmul(
                    ps[:],
                    lhsT=w_sb[:, j, ec * P : (ec + 1) * P],
                    rhs=xT_sb[:, j, c * 512 : (c + 1) * 512],
                    start=(j == 0),
                    stop=(j == DCH - 1),
                )
                yield
            nc.vector.tensor_copy(dst[:, ec, c * 512 : (c + 1) * 512], ps[:])

    def proj_v_gen(c, pool, t16s=None):
        for t16 in (range(4 * c, 4 * c + 4) if t16s is None else t16s):
            psv = pool.tile([P, 512], FP32, name="ps_g")
            for j in range(DCH):
                nc.tensor.matmul(
                    psv[:],
                    lhsT=xT_sb[:, j, t16 * P : (t16 + 1) * P],
                    rhs=wv_sb[:, j, :],
                    start=(j == 0),
                    stop=(j == DCH - 1),
                )
                yield
            nc.vector.tensor_copy(vt[:, t16, :, 0:DK], psv[:])

    # ---- prologue: only the k/q blocks head-pair 0 needs; everything else
    # (rest of chunk-0 k/q/v, later chunks) rides as filler inside the
    # attention stream, guarded by precise drains ----
    with tc.tile_pool(name="pb0", bufs=2, space="PSUM") as pb0:
        for g in (
            proj_qk_gen(wk_sb, kT, 0, pb0, ecs=(0,)),
            proj_qk_gen(wq_sb, qT, 0, pb0, ecs=(0,)),
        ):
            for _ in g:
                pass

    # ---- main fused stream ----
    pt_pool = ctx.enter_context(tc.tile_pool(name="pt", bufs=4))
    lrp = ctx.enter_context(tc.tile_pool(name="lrp", bufs=4))
    obp = ctx.enter_context(tc.tile_pool(name="obp", bufs=2))
    ps_pair = ctx.enter_context(tc.tile_pool(name="ps_pair", bufs=2, space="PSUM"))
    ps_av = ctx.enter_context(tc.tile_pool(name="ps_av", bufs=2, space="PSUM"))
    ps_sh = ctx.enter_context(tc.tile_pool(name="ps_sh", bufs=1, space="PSUM"))
    ps_o = ctx.enter_context(tc.tile_pool(name="ps_o", bufs=1, space="PSUM"))

    # two-tier filler: hi = projections + normalize (dependency-critical),
    # lo = out_proj (drifts into the ACT-bound final chunk)
    proj_gens = {
        c: [
            proj_qk_gen(wk_sb, kT, c, ps_sh),
            proj_v_gen(c, ps_sh),
            proj_qk_gen(wq_sb, qT, c, ps_sh),
        ]
        for c in (1, 2, 3)
    }
    v0_gens = {j: proj_v_gen(0, ps_sh, t16s=(j,)) for j in range(4)}
    kq0_gens = {
        hp: [
            proj_qk_gen(wk_sb, kT, 0, ps_sh, ecs=(hp,)),
            proj_qk_gen(wq_sb, qT, 0, ps_sh, ecs=(hp,)),
        ]
        for hp in (1, 2, 3)
    }
    filler_hi = deque(
        [v0_gens[0], v0_gens[1]] + kq0_gens[1]
        + [v0_gens[2]] + kq0_gens[2]
        + [v0_gens[3]] + kq0_gens[3]
        + proj_gens[1] + proj_gens[2] + proj_gens[3]
    )
    filler_lo = deque()

    norm_done = {}

    def _hi_one():
        if not filler_hi:
            return False
        try:
            next(filler_hi[0])
        except StopIteration:
            filler_hi.popleft()
        return True

    def _lo_one():
        if not filler_lo:
            return False
        cq, g = filler_lo[0]
        if not norm_done.get(cq):  # out_proj must follow its normalize
            return False
        try:
            next(g)
        except StopIteration:
            filler_lo.popleft()
        return True

    def pump(nhi, nlo):
        for _ in range(nhi):
            if not _hi_one() and not _lo_one():
                return
        for _ in range(nlo):
            if not _lo_one() and not _hi_one():
                return

    def drain_gens(gens):
        for g in gens:
            for _ in g:
                pass
            if g in filler_hi:
                filler_hi.remove(g)

    def drain_proj(c):
        drain_gens(proj_gens.get(c, ()))

    def norm_gen(c, rec_bf):
        # attn[:, hp, t1] *= 1/l_h (in place); one K=8 bcast matmul puts both
        # heads' reciprocal rows on partitions 0-63 / 64-127, one DVE multiply
        for hp in range(NH // 2):
            # transient single-bank use of the ps_pair pool: safe because its
            # mult is emitted on the next pump, before the slot can recycle
            psb = ps_pair.tile([P, 2, 512], FP32, name="ps_pair")[:, 0, :]
            nc.tensor.matmul(
                psb,
                lhsT=sel_sb[:, 2 * hp * DK : (2 * hp + 2) * DK],
                rhs=rec_bf[:],
                start=True,
                stop=True,
            )
            yield
            sl = attn[:, hp, c * 512 : (c + 1) * 512]
            nc.vector.tensor_tensor(sl, sl, psb, mybir.AluOpType.mult)
        norm_done[c] = True

    def outproj_gen(c):
        for ti in range(4):
            t0 = c * 512 + ti * P
            ob = obp.tile([P, D], FP32, name="ob")
            for eo in range(2):
                # in the tail (c==3) the AV pool is idle: use it to
                # double-buffer the evacuation chain
                pool, tag = (ps_av, "av") if c == 3 else (ps_o, "ps_o")
                pso = pool.tile([P, 512], FP32, name=tag)[:]
                for s in range(EC):
                    nc.tensor.matmul(
                        pso,
                        lhsT=attn[:, s, t0 : t0 + P],
                        rhs=wo_sb[:, s, eo * 512 : (eo + 1) * 512],
                        start=(s == 0),
                        stop=(s == EC - 1),
                    )
                    yield
                nc.vector.tensor_copy(ob[:, eo * 512 : (eo + 1) * 512], pso)
                nc.sync.dma_start(
                    out[t0 : t0 + P, eo * 512 : (eo + 1) * 512],
                    ob[:, eo * 512 : (eo + 1) * 512],
                )

    cur_l = [None]
    av_tiles = {}
    pend_av = deque()

    def emit_step(c, hp, j):
        m = j - 4 * c
        w = 512 if m < 0 else 512 - m * P
        t1lo = c * 512 + (512 - w)
        pp = ps_pair.tile([P, 2, 512], FP32, name="ps_pair")
        pt2 = pt_pool.tile([P, 2, 512], BF16, name="pt")
        for half, po in ((0, 0), (1, DK)):
            nc.tensor.matmul(
                pp[:, half, 0:w],
                lhsT=kT[po : po + DK, hp, j * P : (j + 1) * P],
                rhs=qT[po : po + DK, hp, t1lo : (c + 1) * 512],
                start=True,
                stop=True,
            )
        if w == 512:
            nc.scalar.activation(
                pt2[:, :, :], pp[:, :, :],
                mybir.ActivationFunctionType.Exp, scale=0.125,
            )
        else:
            for half in (0, 1):
                nc.scalar.activation(
                    pt2[:, half, 0:w], pp[:, half, 0:w],
                    mybir.ActivationFunctionType.Exp, scale=0.125,
                )
        if m >= 0:  # zero the upper triangle of the diagonal 128-block
            sl = pt2[:, :, 0:P]
            nc.vector.tensor_tensor(sl, sl, tri_sb[:], mybir.AluOpType.mult)
        return pt2

    def finish_pair(c, hp):
        if hp == 0:
            cur_l[0] = lrp.tile([NH, 512], FP32, name="lall")
            nc.vector.memset(cur_l[0][:], 0.0)
        for half in (0, 1):
            ps = av_tiles.pop((c, hp, half))
            nc.vector.tensor_copy(
                attn[half * DK : half * DK + DK, hp, c * 512 : (c + 1) * 512],
                ps[0:DK, :],
            )
            # denom rows: this head's l on partition DK+h, zeros elsewhere
            nc.vector.tensor_tensor(
                cur_l[0][:], cur_l[0][:], ps[DK : DK + NH, :],
                mybir.AluOpType.add,
            )
        if hp == NH // 2 - 1:
            rec = lrp.tile([NH, 512], FP32, name="rec")
            nc.vector.reciprocal_approx_fast(rec[:], cur_l[0][:])
            rec_bf = lrp.tile([NH, 512], BF16, name="rec_bf")
            nc.vector.tensor_copy(rec_bf[:], rec[:])
            filler_hi.appendleft(norm_gen(c, rec_bf))
            filler_lo.append((c, outproj_gen(c)))

    def emit_av(c, hp, j, pt2):
        if c == 0:  # vt[j] projection must be emitted before this AV
            drain_gens([v0_gens[j]])
        m = j - 4 * c
        w = 512 if m < 0 else 512 - m * P
        off = 512 - w
        if j == 0:
            av_tiles[(c, hp, 0)] = ps_av.tile([P, 512], FP32, name="av")
            av_tiles[(c, hp, 1)] = ps_av.tile([P, 512], FP32, name="av")
        for half in (0, 1):
            nc.tensor.matmul(
                av_tiles[(c, hp, half)][0:VW, off:512],
                lhsT=vt[:, j, 2 * hp + half, :],
                rhs=pt2[:, half, 0:w],
                start=(j == 0),
                stop=(j == 4 * c + 3),
                skip_group_check=True,
            )
        if j == 4 * c + 3:
            finish_pair(c, hp)

    PUMPS = {0: (4, 0), 1: (3, 1), 2: (3, 1), 3: (1, 3)}
    for c in range(NT512):
        drain_proj(c)
        for hp in range(NH // 2):
            if c == 0 and hp > 0:
                drain_gens(kq0_gens[hp])
            for j in range(4 * c + 4):
                pt2 = emit_step(c, hp, j)
                pend_av.append((c, hp, j, pt2))
                if len(pend_av) > LAG:
                    emit_av(*pend_av.popleft())
                pump(*PUMPS[c])
    while pend_av:
        emit_av(*pend_av.popleft())
    pump(10**9, 10**9)


def _build_nc():
    from contextlib import ExitStack

    nc = bacc.Bacc("TRN2", target_bir_lowering=False, debug=False)
    xT = nc.dram_tensor("xT", [D, T], BF16, kind="ExternalInput")
    wq = nc.dram_tensor("wq", [D, E], BF16, kind="ExternalInput")
    wk = nc.dram_tensor("wk", [D, E], BF16, kind="ExternalInput")
    wv = nc.dram_tensor("wv", [D, E], BF16, kind="ExternalInput")
    wo = nc.dram_tensor("wo", [E, D], BF16, kind="ExternalInput")
    tri = nc.dram_tensor("tri", [P, 2, P], BF16, kind="ExternalInput")
    sel = nc.dram_tensor("sel", [NH, NH * DK], BF16, kind="ExternalInput")
    out = nc.dram_tensor("out", [T, D], FP32, kind="ExternalOutput")

    with (
        tile.TileContext(nc) as tc,
        nc.allow_low_precision(reason="bf16 intermediates by design"),
        ExitStack() as ctx,
    ):
        _emit(
            nc, tc, ctx, xT[:], wq[:], wk[:], wv[:], wo[:], tri[:], sel[:], out[:]
        )
    nc.compile()
    return nc


def _host_inputs(x, W_qkv, W_o):
    x = np.asarray(x, dtype=np.float32)
    W_qkv = np.asarray(W_qkv, dtype=np.float32)
    W_o = np.asarray(W_o, dtype=np.float32)
    bf = ml_dtypes.bfloat16
    # tri[t2, :, t1] = 1 where t1 >= t2 (keep), 0 on above-diagonal columns;
    # duplicated on axis 1 so one DVE op masks both heads' tiles
    t1m = np.triu(np.ones((P, P)))
    tri = np.ascontiguousarray(np.stack([t1m, t1m], axis=1)).astype(bf)
    sel = np.zeros((NH, NH * DK), dtype=np.float32)
    for hh in range(NH):
        sel[hh, hh * DK : (hh + 1) * DK] = 1.0
    sel = sel.astype(bf)
    in_maps = []
    for b in range(B):
        xTb = np.ascontiguousarray(x[b].T.astype(bf))
        for hg in range(2):
            sl = slice(E * hg, E * hg + E)
            in_maps.append(
                {
                    "xT": xTb,
                    "wq": np.ascontiguousarray(W_qkv[0 * D :][sl].T.astype(bf)),
                    "wk": np.ascontiguousarray(W_qkv[1 * D :][sl].T.astype(bf)),
                    "wv": np.ascontiguousarray(W_qkv[2 * D :][sl].T.astype(bf)),
                    "wo": np.ascontiguousarray(W_o[:, sl].T.astype(bf)),
                    "tri": tri,
                    "sel": sel,
                }
            )
    return in_maps


def _run(x, W_qkv, W_o, trace=False, tmpdir=None):
    nc = _build_nc()
    in_maps = _host_inputs(x, W_qkv, W_o)
    res = run_bass_kernel_spmd(
        nc, in_maps, list(range(NCORES)), trace=trace, tmpdir=tmpdir
    )
    out = np.empty((B, T, D), dtype=np.float32)
    for b in range(B):
        out[b] = res.results[2 * b]["out"] + res.results[2 * b + 1]["out"]
    return out, res.exec_time_ns


def kernel(x, W_qkv, W_o):
    out, _ = _run(x, W_qkv, W_o, trace=False)
    return out


# revision 55
# speedup vs baseline: 1.0342x; 1.0342x over previous
"""Causal multi-head self-attention on 8 TRN2 NeuronCores.

Problem (hardcoded): x (4, 2048, 1024) f32, W_qkv (3072, 1024), W_o (1024, 1024).
  qkv = x @ W_qkv.T; q,k,v split -> (B,H,T,DK) with H=16, DK=64
  scores = q k^T / 8 + causal mask; attn = softmax; out = (attn v) @ W_o.T
Sharding: core = 2*b + hg (b in 0..3, hg in 0..1 head-groups of 8 heads).
Each core computes a partial out[b] over its 512 attn columns; host sums pairs.

v2 dataflow — one fused instruction stream designed to keep the PE array
HAM-warm (no idle windows) and hide the ACT exp stream:
  - Heads are processed in even/odd pairs. Even heads live on SBUF
    partitions 0-63, odd heads on 64-127, so the two K=64 S^T matmuls of a
    pair auto-row-tile to (0,0)/(64,0) and run concurrently, writing the
    two banks of one [128,1024] PSUM tile; a single wide exp covers both.
  - Causal mask: exp only covers valid widths; the 128-wide diagonal block
    is zeroed post-exp by a bf16 triangular multiply in SBUF (fast DVE mode)
    instead of f32 PSUM adds.
  - QKV projections are computed per 512-wide t-chunk and interleaved into
    the attention stream as PE "filler" (chunks 0-1 in a prologue, later
    chunks funded by earlier attention chunks), so the ACT-bound exp stream
    never leaves the PE idle.
  - AV accumulates v^T x P^T with an extra ones-column per head producing
    softmax denominators on PSUM partitions 64+h; evacuated per pair with a
    lag that crosses pair boundaries (3-deep AV PSUM pool).
  - Normalize: reciprocal_approx_fast on the chunk's [8,512] denominators,
    bf16 K=8 PE broadcast matmul, in-place bf16 DVE multiply. Normalize jobs
    go in the high-priority filler tier; out_proj jobs (attn^T x W_o slices,
    f32 partials to DRAM) in the low tier so they drift into the ACT-bound
    final chunk.
"""

import sys

import numpy as np

sys.path.insert(0, "/opt/trn_rl_repo")

import ml_dtypes  # noqa: E402

from concourse import bacc, bass, mybir, tile  # noqa: E402
from concourse.bass_utils import run_bass_kernel_spmd  # noqa: E402

FP32 = mybir.dt.float32
BF16 = mybir.dt.bfloat16

B, T, D, H, DK = 4, 2048, 1024, 16, 64
NCORES = 8
E = 512          # qkv columns per head-group
NH = 8           # heads per core
P = 128
DCH = D // P     # 8 contraction chunks for the projections
EC = E // P      # 4 e-chunks (head pairs)
NT512 = T // 512
NT128 = T // P
VW = DK + NH     # 72: AV output rows (64 values + 8 one-hot denom rows)
LAG = 3          # AV stream lags the S/exp stream by this many j-steps


def _emit(nc, tc, ctx, xT, wq, wk, wv, wo, tri, sel, out):
    from collections import deque

    consts = ctx.enter_context(tc.tile_pool(name="consts", bufs=1))
    persist = ctx.enter_context(tc.tile_pool(name="persist", bufs=1))

    # ---- persistent SBUF state ----
    xT_sb = persist.tile([P, DCH, T], BF16, name="xT_sb")
    wq_sb = persist.tile([P, DCH, E], BF16, name="wq_sb")
    wk_sb = persist.tile([P, DCH, E], BF16, name="wk_sb")
    wv_sb = persist.tile([P, DCH, E], BF16, name="wv_sb")
    wo_sb = persist.tile([P, EC, D], BF16, name="wo_sb")
    qT = persist.tile([P, EC, T], BF16, name="qT")        # e = ec*128+p
    kT = persist.tile([P, EC, T], BF16, name="kT")
    vt = persist.tile([P, NT128, NH, VW], BF16, name="vt")
    attn = persist.tile([P, EC, T], BF16, name="attn")    # dl = s*128+p
    tri_sb = consts.tile([P, 2, P], BF16, name="tri_sb")
    sel_sb = consts.tile([NH, NH * DK], BF16, name="sel_sb")

    # ---- input DMA: x chunks on sync, weights on gpsimd, late x on scalar,
    # so three DMA streams run in parallel and k-proj deps arrive fastest ----
    for j in range(DCH):
        nc.sync.dma_start(
            xT_sb[:, j, 0:512], xT[j * P : (j + 1) * P, 0:512]
        )
        nc.gpsimd.dma_start(wk_sb[:, j], wk[j * P : (j + 1) * P, :])
    for j in range(DCH):
        nc.gpsimd.dma_start(wq_sb[:, j], wq[j * P : (j + 1) * P, :])
    for j in range(DCH):
        nc.sync.dma_start(
            xT_sb[:, j, 512:1024], xT[j * P : (j + 1) * P, 512:1024]
        )
        nc.gpsimd.dma_start(wv_sb[:, j], wv[j * P : (j + 1) * P, :])
    nc.gpsimd.dma_start(tri_sb[:], tri[:])
    nc.gpsimd.dma_start(sel_sb[:], sel[:])
    for c in (2, 3):
        for j in range(DCH):
            nc.scalar.dma_start(
                xT_sb[:, j, c * 512 : (c + 1) * 512],
                xT[j * P : (j + 1) * P, c * 512 : (c + 1) * 512],
            )
    for s in range(EC):
        nc.gpsimd.dma_start(wo_sb[:, s], wo[s * P : (s + 1) * P, :])

    # head h's ones column sits at DK+h so its denominators land on a
    # distinct psum partition (64+h); other heads' columns there are zero
    nc.vector.memset(vt[:, :, :, DK:], 0.0)
    for hh in range(NH):
        nc.vector.memset(vt[:, :, hh, DK + hh], 1.0)

    # ---- projection generators (each yield ~= one matmul of PE work) ----
    def proj_qk_gen(w_sb, dst, c, pool, ecs=None):
        for ec in (range(EC) if ecs is None else ecs):
            ps = pool.tile([P, 512], FP32, name="ps_g")
            for j in range(DCH):
                nc.tensor.matmul(
                    ps[:],
                    lhsT=w_sb[:, j, ec * P : (ec + 1) * P],
                    rhs=xT_sb[:, j, c * 512 : (c + 1) * 512],
                    start=(j == 0),
                    stop=(j == DCH - 1),
                )
                yield
            nc.vector.tensor_copy(dst[:, ec, c * 512 : (c + 1) * 512], ps[:])

    def proj_v_gen(c, pool, t16s=None):
        for t16 in (range(4 * c, 4 * c + 4) if t16s is None else t16s):
            psv = pool.tile([P, 512], FP32, name="ps_g")
            for j in range(DCH):
                nc.tensor.matmul(
                    psv[:],
                    lhsT=xT_sb[:, j, t16 * P : (t16 + 1) * P],
                    rhs=wv_sb[:, j, :],
                    start=(j == 0),
                    stop=(j == DCH - 1),
                )
                yield
            nc.vector.tensor_copy(vt[:, t16, :, 0:DK], psv[:])

    # ---- prologue: k/q of chunk 0 plus the first v blocks; the rest of v
    # and all later chunks ride as filler inside the attention stream ----
    with tc.tile_pool(name="pb0", bufs=2, space="PSUM") as pb0:
        for g in (
            proj_qk_gen(wk_sb, kT, 0, pb0),
            proj_qk_gen(wq_sb, qT, 0, pb0),
            proj_v_gen(0, pb0, t16s=(0, 1, 2)),
        ):
            for _ in g:
                pass

    # ---- main fused stream ----
    pt_pool = ctx.enter_context(tc.tile_pool(name="pt", bufs=4))
    lrp = ctx.enter_context(tc.tile_pool(name="lrp", bufs=4))
    obp = ctx.enter_context(tc.tile_pool(name="obp", bufs=2))
    ps_pair = ctx.enter_context(tc.tile_pool(name="ps_pair", bufs=2, space="PSUM"))
    ps_av = ctx.enter_context(tc.tile_pool(name="ps_av", bufs=2, space="PSUM"))
    ps_sh = ctx.enter_context(tc.tile_pool(name="ps_sh", bufs=1, space="PSUM"))
    ps_o = ctx.enter_context(tc.tile_pool(name="ps_o", bufs=1, space="PSUM"))

    # two-tier filler: hi = projections + normalize (dependency-critical),
    # lo = out_proj (drifts into the ACT-bound final chunk)
    proj_gens = {
        c: [
            proj_qk_gen(wk_sb, kT, c, ps_sh),
            proj_v_gen(c, ps_sh),
            proj_qk_gen(wq_sb, qT, c, ps_sh),
        ]
        for c in (1, 2, 3)
    }
    proj_gens[1].insert(0, proj_v_gen(0, ps_sh, t16s=(3,)))
    filler_hi = deque(proj_gens[1] + proj_gens[2] + proj_gens[3])
    filler_lo = deque()

    norm_done = {}

    def _hi_one():
        if not filler_hi:
            return False
        try:
            next(filler_hi[0])
        except StopIteration:
            filler_hi.popleft()
        return True

    def _lo_one():
        if not filler_lo:
            return False
        cq, g = filler_lo[0]
        if not norm_done.get(cq):  # out_proj must follow its normalize
            return False
        try:
            next(g)
        except StopIteration:
            filler_lo.popleft()
        return True

    def pump(nhi, nlo):
        for _ in range(nhi):
            if not _hi_one() and not _lo_one():
                return
        for _ in range(nlo):
            if not _lo_one() and not _hi_one():
                return

    def drain_gens(gens):
        for g in gens:
            for _ in g:
                pass
            if g in filler_hi:
                filler_hi.remove(g)

    def drain_proj(c):
        drain_gens(proj_gens.get(c, ()))

    def norm_gen(c, rec_bf):
        # attn[:, hp, t1] *= 1/l_h (in place); one K=8 bcast matmul puts both
        # heads' reciprocal rows on partitions 0-63 / 64-127, one DVE multiply
        for hp in range(NH // 2):
            # transient single-bank use of the ps_pair pool: safe because its
            # mult is emitted on the next pump, before the slot can recycle
            psb = ps_pair.tile([P, 2, 512], FP32, name="ps_pair")[:, 0, :]
            nc.tensor.matmul(
                psb,
                lhsT=sel_sb[:, 2 * hp * DK : (2 * hp + 2) * DK],
                rhs=rec_bf[:],
                start=True,
                stop=True,
            )
            yield
            sl = attn[:, hp, c * 512 : (c + 1) * 512]
            nc.vector.tensor_tensor(sl, sl, psb, mybir.AluOpType.mult)
        norm_done[c] = True

    def outproj_gen(c):
        for ti in range(4):
            t0 = c * 512 + ti * P
            ob = obp.tile([P, D], FP32, name="ob")
            for eo in range(2):
                # in the tail (c==3) the AV pool is idle: use it to
                # double-buffer the evacuation chain
                pool, tag = (ps_av, "av") if c == 3 else (ps_o, "ps_o")
                pso = pool.tile([P, 512], FP32, name=tag)[:]
                for s in range(EC):
                    nc.tensor.matmul(
                        pso,
                        lhsT=attn[:, s, t0 : t0 + P],
                        rhs=wo_sb[:, s, eo * 512 : (eo + 1) * 512],
                        start=(s == 0),
                        stop=(s == EC - 1),
                    )
                    yield
                nc.vector.tensor_copy(ob[:, eo * 512 : (eo + 1) * 512], pso)
                nc.sync.dma_start(
                    out[t0 : t0 + P, eo * 512 : (eo + 1) * 512],
                    ob[:, eo * 512 : (eo + 1) * 512],
                )

    cur_l = [None]
    av_tiles = {}
    pend_av = deque()

    def emit_step(c, hp, j):
        m = j - 4 * c
        w = 512 if m < 0 else 512 - m * P
        t1lo = c * 512 + (512 - w)
        pp = ps_pair.tile([P, 2, 512], FP32, name="ps_pair")
        pt2 = pt_pool.tile([P, 2, 512], BF16, name="pt")
        for half, po in ((0, 0), (1, DK)):
            nc.tensor.matmul(
                pp[:, half, 0:w],
                lhsT=kT[po : po + DK, hp, j * P : (j + 1) * P],
                rhs=qT[po : po + DK, hp, t1lo : (c + 1) * 512],
                start=True,
                stop=True,
            )
        if w == 512:
            nc.scalar.activation(
                pt2[:, :, :], pp[:, :, :],
                mybir.ActivationFunctionType.Exp, scale=0.125,
            )
        else:
            for half in (0, 1):
                nc.scalar.activation(
                    pt2[:, half, 0:w], pp[:, half, 0:w],
                    mybir.ActivationFunctionType.Exp, scale=0.125,
                )
        if m >= 0:  # zero the upper triangle of the diagonal 128-block
            sl = pt2[:, :, 0:P]
            nc.vector.tensor_tensor(sl, sl, tri_sb[:], mybir.AluOpType.mult)
        return pt2

    def finish_pair(c, hp):
        if hp == 0:
            cur_l[0] = lrp.tile([NH, 512], FP32, name="lall")
            nc.vector.memset(cur_l[0][:], 0.0)
        for half in (0, 1):
            ps = av_tiles.pop((c, hp, half))
            nc.vector.tensor_copy(
                attn[half * DK : half * DK + DK, hp, c * 512 : (c + 1) * 512],
                ps[0:DK, :],
            )
            # denom rows: this head's l on partition DK+h, zeros elsewhere
            nc.vector.tensor_tensor(
                cur_l[0][:], cur_l[0][:], ps[DK : DK + NH, :],
                mybir.AluOpType.add,
            )
        if hp == NH // 2 - 1:
            rec = lrp.tile([NH, 512], FP32, name="rec")
            nc.vector.reciprocal_approx_fast(rec[:], cur_l[0][:])
            rec_bf = lrp.tile([NH, 512], BF16, name="rec_bf")
            nc.vector.tensor_copy(rec_bf[:], rec[:])
            filler_hi.appendleft(norm_gen(c, rec_bf))
            filler_lo.append((c, outproj_gen(c)))

    def emit_av(c, hp, j, pt2):
        m = j - 4 * c
        w = 512 if m < 0 else 512 - m * P
        off = 512 - w
        if j == 0:
            av_tiles[(c, hp, 0)] = ps_av.tile([P, 512], FP32, name="av")
            av_tiles[(c, hp, 1)] = ps_av.tile([P, 512], FP32, name="av")
        for half in (0, 1):
            nc.tensor.matmul(
                av_tiles[(c, hp, half)][0:VW, off:512],
                lhsT=vt[:, j, 2 * hp + half, :],
                rhs=pt2[:, half, 0:w],
                start=(j == 0),
                stop=(j == 4 * c + 3),
                skip_group_check=True,
            )
        if j == 4 * c + 3:
            finish_pair(c, hp)

    PUMPS = {0: (3, 0), 1: (3, 1), 2: (3, 1), 3: (1, 2)}
    for c in range(NT512):
        drain_proj(c)
        for hp in range(NH // 2):
            for j in range(4 * c + 4):
                pt2 = emit_step(c, hp, j)
                pend_av.append((c, hp, j, pt2))
                if len(pend_av) > LAG:
                    emit_av(*pend_av.popleft())
                pump(*PUMPS[c])
    while pend_av:
        emit_av(*pend_av.popleft())
    pump(10**9, 10**9)


def _build_nc():
    from contextlib import ExitStack

    nc = bacc.Bacc("TRN2", target_bir_lowering=False, debug=False)
    xT = nc.dram_tensor("xT", [D, T], BF16, kind="ExternalInput")
    wq = nc.dram_tensor("wq", [D, E], BF16, kind="ExternalInput")
    wk = nc.dram_tensor("wk", [D, E], BF16, kind="ExternalInput")
    wv = nc.dram_tensor("wv", [D, E], BF16, kind="ExternalInput")
    wo = nc.dram_tensor("wo", [E, D], BF16, kind="ExternalInput")
    tri = nc.dram_tensor("tri", [P, 2, P], BF16, kind="ExternalInput")
    sel = nc.dram_tensor("sel", [NH, NH * DK], BF16, kind="ExternalInput")
    out = nc.dram_tensor("out", [T, D], FP32, kind="ExternalOutput")

    with (
        tile.TileContext(nc) as tc,
        nc.allow_low_precision(reason="bf16 intermediates by design"),
        ExitStack() as ctx,
    ):
        _emit(
            nc, tc, ctx, xT[:], wq[:], wk[:], wv[:], wo[:], tri[:], sel[:], out[:]
        )
    nc.compile()
    return nc


def _host_inputs(x, W_qkv, W_o):
    x = np.asarray(x, dtype=np.float32)
    W_qkv = np.asarray(W_qkv, dtype=np.float32)
    W_o = np.asarray(W_o, dtype=np.float32)
    bf = ml_dtypes.bfloat16
    # tri[t2, :, t1] = 1 where t1 >= t2 (keep), 0 on above-diagonal columns;
    # duplicated on axis 1 so one DVE op masks both heads' tiles
    t1m = np.triu(np.ones((P, P)))
    tri = np.ascontiguousarray(np.stack([t1m, t1m], axis=1)).astype(bf)
    sel = np.zeros((NH, NH * DK), dtype=np.float32)
    for hh in range(NH):
        sel[hh, hh * DK : (hh + 1) * DK] = 1.0
    sel = sel.astype(bf)
    in_maps = []
    for b in range(B):
        xTb = np.ascontiguousarray(x[b].T.astype(bf))
        for hg in range(2):
            sl = slice(E * hg, E * hg + E)
            in_maps.append(
                {
                    "xT": xTb,
                    "wq": np.ascontiguousarray(W_qkv[0 * D :][sl].T.astype(bf)),
                    "wk": np.ascontiguousarray(W_qkv[1 * D :][sl].T.astype(bf)),
                    "wv": np.ascontiguousarray(W_qkv[2 * D :][sl].T.astype(bf)),
                    "wo": np.ascontiguousarray(W_o[:, sl].T.astype(bf)),
                    "tri": tri,
                    "sel": sel,
                }
            )
    return in_maps


def _run(x, W_qkv, W_o, trace=False, tmpdir=None):
    nc = _build_nc()
    in_maps = _host_inputs(x, W_qkv, W_o)
    res = run_bass_kernel_spmd(
        nc, in_maps, list(range(NCORES)), trace=trace, tmpdir=tmpdir
    )
    out = np.empty((B, T, D), dtype=np.float32)
    for b in range(B):
        out[b] = res.results[2 * b]["out"] + res.results[2 * b + 1]["out"]
    return out, res.exec_time_ns


def kernel(x, W_qkv, W_o):
    out, _ = _run(x, W_qkv, W_o, trace=False)
    return out
